# revision 6
# baseline (speedup 1.0000x reference)
"""MLPConv (3x3 valid conv -> 256 -> 256 MLP with ReLU) on 8 TRN2 cores.

Data-parallel over batch: 4 images per core. Per image, the conv is
computed as 9 PSUM-accumulated matmuls (one per filter tap) contracting
over C=128 on the partition dim, with the input transposed on the PE
(identity matmul) into [C, H*W] layout. Both MLP stages keep the
[F, pixels] transposed layout so stage-2 consumes stage-1's output
directly and the per-partition bias lands on the ACT engine's bias port.
Matmuls run as float32r (replicated fp32, 1 cycle/row at N>=256).

Output per core is [F_half, f, img, 62*64 grid]; the host slices the
valid 62 columns and assembles the [F, N, B]-ordered buffer that the
reference reinterprets as [B, 62, 62, F].
"""

import numpy as np

import concourse.bass as bass
import concourse.mybir as mybir
import concourse.tile as tile
from concourse.bass_utils import run_bass_kernel_spmd
from concourse.masks import make_identity

B, H, W, C = 32, 64, 64, 128
F = 256
N_CORES = 8
IMG_PER_CORE = B // N_CORES
HW = H * W                      # 4096 input pixels per image
GRID = 62 * 64                  # 3968 output-grid pixels (64-wide, 62 rows)
NBLK = 8
BLK = GRID // NBLK              # 496 <= 512 fp32 moving-dim limit
XT_PAD = HW + 2 * W + 2         # moving slices reach index 4097

F32 = mybir.dt.float32
F32R = mybir.dt.float32r
BF16 = mybir.dt.bfloat16
RELU = mybir.ActivationFunctionType.Relu


def _split_multi_waits(nc):
    """This container's walrus rejects >1 semaphore wait per instruction
    ("Too many sync wait commands"). Move all but the last wait of each
    instruction onto single-wait NoOps right before it on the same engine."""
    n = 0
    for f in nc.m.functions:
        for bb in f.blocks:
            insts = bb.instructions
            if not any(
                i.sync_info is not None and len(i.sync_info.on_wait) > 1
                for i in insts
            ):
                continue
            new_insts = []
            for inst in insts:
                si = inst.sync_info
                if si is not None and len(si.on_wait) > 1:
                    waits = list(si.on_wait)
                    for k, w in enumerate(waits[:-1]):
                        new_insts.append(
                            mybir.InstNoOp(
                                name=f"{inst.name}-wsplit{k}",
                                engine=inst.engine,
                                bass_nofuse=True,
                                sync_info=mybir.SyncInfo(on_wait=[w], on_update=[]),
                            )
                        )
                        n += 1
                    inst.sync_info = mybir.SyncInfo(
                        on_wait=[waits[-1]], on_update=list(si.on_update)
                    )
                new_insts.append(inst)
            bb.instructions = new_insts
    return n


def build_nc():
    nc = bass.Bass("TRN2", target_bir_lowering=False)
    x = nc.dram_tensor("x", [IMG_PER_CORE, HW, C], F32, kind="ExternalInput").ap()
    w0 = nc.dram_tensor("w0", [9 * C, F], F32, kind="ExternalInput").ap()
    b0 = nc.dram_tensor("b0", [F], F32, kind="ExternalInput").ap()
    w1 = nc.dram_tensor("w1", [F, F], F32, kind="ExternalInput").ap()
    b1 = nc.dram_tensor("b1", [F], F32, kind="ExternalInput").ap()
    out = nc.dram_tensor(
        "out", [2, 128, IMG_PER_CORE, GRID], F32, kind="ExternalOutput"
    ).ap()

    with tile.TileContext(nc) as tc:
        with (
            tc.tile_pool(name="consts", bufs=1) as consts,
            tc.tile_pool(name="xl", bufs=6) as xl,
            tc.tile_pool(name="xlb", bufs=6) as xlbp,
            tc.tile_pool(name="xT", bufs=2) as xT,  # two half-image tiles per img
            tc.tile_pool(name="h1T", bufs=4) as h1T,
            tc.tile_pool(name="outb", bufs=4) as outb,
            tc.tile_pool(name="pt", bufs=2, space="PSUM") as pt,
            tc.tile_pool(name="ps1", bufs=4, space="PSUM") as ps1,
            tc.tile_pool(name="ps2", bufs=2, space="PSUM") as ps2,
        ):
            ident = consts.tile([128, 128], BF16)
            make_identity(nc, ident)

            # first image's input DMAs go first so the PE can start promptly
            xls = {}
            for j in range(4):
                xlt = xl.tile([128, 8, 128], F32, name="xlt")
                nc.sync.dma_start(
                    xlt[:], x[0].rearrange("(b p) c -> p b c", p=128)[:, 8 * j : 8 * (j + 1), :]
                )
                xls[(0, j)] = xlt

            w0f = consts.tile([128, 9, F], F32)
            nc.sync.dma_start(w0f[:], w0.rearrange("(t c) f -> c t f", c=128))
            w0b = consts.tile([128, 9, F], BF16)
            nc.vector.tensor_copy(w0b[:], w0f[:])
            w1f = consts.tile([128, 2, F], F32)
            nc.sync.dma_start(w1f[:], w1.rearrange("(k c) f -> c k f", c=128))
            w1s = consts.tile([128, 2, F], F32R)
            nc.vector.tensor_copy(w1s[:], w1f[:])
            b0s = consts.tile([128, 2], F32)
            nc.sync.dma_start(b0s[:], b0.rearrange("(h f) -> f h", f=128))
            b1s = consts.tile([128, 2], F32)
            nc.sync.dma_start(b1s[:], b1.rearrange("(h f) -> f h", f=128))

            for img in range(IMG_PER_CORE):
                ximg = x[img].rearrange("(b p) c -> p b c", p=128)  # [128, 32, 128]
                # Half-image xT tiles: stage-1 blocks 0-3 only need input px
                # 0..2113 (transposes 0..16), so they can start while the
                # second half of the image is still loading/transposing.
                xTa = xT.tile([128, 17 * 128], BF16, name="xTa")
                xTb = xT.tile([128, 18 * 128], BF16, name="xTb")  # px 1920.. + pad
                xlbs = []
                for j in range(4):
                    if (img, j) in xls:
                        xlt = xls.pop((img, j))
                    else:
                        xlt = xl.tile([128, 8, 128], F32, name="xlt")
                        nc.sync.dma_start(xlt[:], ximg[:, 8 * j : 8 * (j + 1), :])
                    xlb = xlbp.tile([128, 8, 128], BF16, name="xlb")
                    nc.vector.tensor_copy(xlb[:], xlt[:])
                    xlbs.append(xlb)
                for dst, p0, plist in (
                    (xTa, 0, range(0, 17)),
                    (xTb, 15, range(15, 32)),
                ):
                    for b0i in range(0, 17, 4):
                        batch = list(plist)[b0i : b0i + 4]
                        nb = len(batch)
                        ptt = pt.tile([128, 4, 128], BF16, name="ptt")
                        for q, p in enumerate(batch):
                            nc.tensor.transpose(
                                ptt[:, q, :], xlbs[p // 8][:, p % 8, :], ident[:]
                            )
                        nc.vector.tensor_copy(
                            dst[:, 128 * (batch[0] - p0) : 128 * (batch[0] - p0 + nb)],
                            ptt[:, :nb, :].rearrange("p a b -> p (a b)"),
                        )

                h1 = []
                for h in range(2):
                    h1t = h1T.tile([128, GRID], F32R, name="h1t")
                    h1.append(h1t)
                for part, xpart, base in ((0, xTa, 0), (1, xTb, 1920)):
                    for h in range(2):
                        pss = []
                        for bi in range(4):
                            ps1t = ps1.tile([128, BLK], F32, name="ps1t")
                            pss.append(ps1t)
                        for t in range(9):
                            off = (t // 3) * W + (t % 3)
                            wtap = w0b[:, t, 128 * h : 128 * (h + 1)]
                            for bi in range(4):
                                s = (4 * part + bi) * BLK + off - base
                                nc.tensor.matmul(
                                    pss[bi][:],
                                    wtap,
                                    xpart[:, s : s + BLK],
                                    start=(t == 0),
                                    stop=(t == 8),
                                )
                        for bi in range(4):
                            s = (4 * part + bi) * BLK
                            if h == 0:
                                nc.scalar.activation(
                                    h1[h][:, s : s + BLK],
                                    pss[bi][:],
                                    RELU,
                                    bias=b0s[:, h : h + 1],
                                )
                            else:
                                nc.vector.tensor_scalar(
                                    out=h1[h][:, s : s + BLK],
                                    in0=pss[bi][:],
                                    scalar1=b0s[:, h : h + 1],
                                    scalar2=0.0,
                                    op0=mybir.AluOpType.add,
                                    op1=mybir.AluOpType.max,
                                )

                for h in range(2):
                    for blk in range(NBLK):
                        s = blk * BLK
                        ps2t = ps2.tile([128, BLK], F32)
                        for k in range(2):
                            nc.tensor.matmul(
                                ps2t[:],
                                w1s[:, k, 128 * h : 128 * (h + 1)],
                                h1[k][:, s : s + BLK],
                                start=(k == 0),
                                stop=(k == 1),
                            )
                        ot = outb.tile([128, BLK], F32)
                        if h == 0:
                            nc.scalar.activation(
                                ot[:], ps2t[:], RELU, bias=b1s[:, h : h + 1]
                            )
                        else:
                            nc.vector.tensor_scalar(
                                out=ot[:],
                                in0=ps2t[:],
                                scalar1=b1s[:, h : h + 1],
                                scalar2=0.0,
                                op0=mybir.AluOpType.add,
                                op1=mybir.AluOpType.max,
                            )
                        nc.sync.dma_start(out[h, :, img, s : s + BLK], ot[:])

    _split_multi_waits(nc)
    return nc


_NC_CACHE = None


def kernel(inputs, w0, b0, w1, b1):
    global _NC_CACHE
    x = np.ascontiguousarray(np.asarray(inputs, dtype=np.float32))
    w0 = np.ascontiguousarray(np.asarray(w0, dtype=np.float32))
    w1 = np.ascontiguousarray(np.asarray(w1, dtype=np.float32))
    b0 = np.ascontiguousarray(np.asarray(b0, dtype=np.float32))
    b1 = np.ascontiguousarray(np.asarray(b1, dtype=np.float32))

    if _NC_CACHE is None:
        _NC_CACHE = build_nc()
    nc = _NC_CACHE

    in_maps = [
        {
            "x": x[c * IMG_PER_CORE : (c + 1) * IMG_PER_CORE].reshape(
                IMG_PER_CORE, HW, C
            ),
            "w0": w0,
            "b0": b0,
            "w1": w1,
            "b1": b1,
        }
        for c in range(N_CORES)
    ]
    res = run_bass_kernel_spmd(nc, in_maps, core_ids=list(range(N_CORES)))

    final = np.empty((B, 62, 62, F), np.float32)
    vf = final.reshape(F, 62 * 62, B)  # the [F, N, B] view the reference reshapes
    for c in range(N_CORES):
        oc = res.results[c]["out"].reshape(F, IMG_PER_CORE, 62, 64)
        oc = oc[:, :, :, :62].reshape(F, IMG_PER_CORE, 62 * 62)
        for i in range(IMG_PER_CORE):
            vf[:, :, c * IMG_PER_CORE + i] = oc[:, i]
    return final
